# revision 9
# baseline (speedup 1.0000x reference)
import os
import sys

for _p in ("/opt/trn_rl_repo", "/root/.axon_site/_ro/trn_rl_repo"):
    if os.path.isdir(_p) and _p not in sys.path:
        sys.path.insert(0, _p)

import numpy as np
import ml_dtypes

import concourse.bass as bass
import concourse.tile as tile
from concourse import bacc, mybir
from concourse.bass_utils import run_bass_kernel_spmd
from concourse.masks import make_identity

F32 = mybir.dt.float32
BF16 = mybir.dt.bfloat16
I16 = mybir.dt.int16
AF = mybir.ActivationFunctionType
ALU = mybir.AluOpType

# ---- problem constants (hardcoded per contract) ----
N_NODES = 10000
N_EDGES = 160000
C = 16            # channels
Q = 5             # 2*order+1
CQ = 80           # C*Q
NF = 10           # N_FREQ * N_RINGS
OP = 80           # output C*Q
NS = 7            # nonlin regular samples
ORDER = 2

NCORES = 8
SLAB = 1280                    # nodes per core (10240 padded)
NPAD = NCORES * SLAB           # 10240
GW = 4                         # nodes per group (one 128-edge tile per group)
T = SLAB // GW                 # 320 tiles per core
NIDX = T * 128                 # 40960 gather slots per core
NT = SLAB // 128               # 10 node tiles per core
GPNT = 128 // GW               # 32 groups per node tile
CHT = 64                       # tiles per gather chunk
NCHUNK = T // CHT              # 5 chunks
WIN = NF * GW                  # 40-wide poh window

_CACHE = {}


def _build_program():
    nc = bacc.Bacc("TRN2", target_bir_lowering=False, debug=False, num_devices=NCORES)

    xpad_d = nc.dram_tensor("xpad", [NPAD, 128], BF16, kind="ExternalInput")
    idx_d = nc.dram_tensor("idx", [128, NIDX // 16], I16, kind="ExternalInput")
    poh_d = nc.dram_tensor("poh", [128, T, WIN], BF16, kind="ExternalInput")
    phi_d = nc.dram_tensor("phi", [128, T], F32, kind="ExternalInput")
    w1_d = nc.dram_tensor("w1", [CQ, NF, OP], BF16, kind="ExternalInput")
    w2_d = nc.dram_tensor("w2", [CQ, NF, OP], BF16, kind="ExternalInput")
    ws1_d = nc.dram_tensor("ws1", [CQ + 1, OP], BF16, kind="ExternalInput")
    ws2_d = nc.dram_tensor("ws2", [CQ + 1, OP], BF16, kind="ExternalInput")
    ipad_d = nc.dram_tensor("ipad", [CQ + 1, OP], BF16, kind="ExternalInput")
    bd1_d = nc.dram_tensor("bd1", [CQ, C * NS], BF16, kind="ExternalInput")
    bd2_d = nc.dram_tensor("bd2", [C * NS, OP], BF16, kind="ExternalInput")
    xslabt_d = nc.dram_tensor("xslabt", [CQ + 1, SLAB], BF16, kind="ExternalInput")
    out_d = nc.dram_tensor("out", [SLAB, OP], F32, kind="ExternalOutput")

    with tile.TileContext(nc) as tc:
        _emit(nc, tc, xpad_d, idx_d, poh_d, phi_d, w1_d, w2_d, ws1_d, ws2_d,
              ipad_d, bd1_d, bd2_d, xslabt_d, out_d)
    nc.compile()
    return nc


def _emit(nc, tc, xpad_d, idx_d, poh_d, phi_d, w1_d, w2_d, ws1_d, ws2_d,
          ipad_d, bd1_d, bd2_d, xslabt_d, out_d):
    from contextlib import ExitStack
    ctx = ExitStack()
    setup = ctx.enter_context(tc.tile_pool(name="setup", bufs=1))
    dram = ctx.enter_context(tc.tile_pool(name="dram", bufs=1, space="DRAM"))
    gpool = ctx.enter_context(tc.tile_pool(name="gather", bufs=2))
    xtpool = ctx.enter_context(tc.tile_pool(name="xt", bufs=2))
    tmppool = ctx.enter_context(tc.tile_pool(name="tmp", bufs=2))
    Gpool = ctx.enter_context(tc.tile_pool(name="G", bufs=2, space="PSUM"))
    npsum = ctx.enter_context(tc.tile_pool(name="npsum", bufs=2, space="PSUM"))
    nsb = ctx.enter_context(tc.tile_pool(name="nsb", bufs=2))

    # ---- persistent inputs in SBUF ----
    idx = setup.tile([128, NIDX // 16], I16)
    poh = setup.tile([128, T, WIN], BF16)
    phi = setup.tile([128, T], F32)
    w1 = setup.tile([CQ, NF, OP], BF16)
    w2 = setup.tile([CQ, NF, OP], BF16)
    ws1 = setup.tile([CQ + 1, OP], BF16)
    ws2 = setup.tile([CQ + 1, OP], BF16)
    ipad = setup.tile([CQ + 1, OP], BF16)
    bd1 = setup.tile([CQ, C * NS], BF16)
    bd2 = setup.tile([C * NS, OP], BF16)
    xslabt = setup.tile([CQ + 1, SLAB], BF16)
    for t_, d_ in ((idx, idx_d), (poh, poh_d), (phi, phi_d), (w1, w1_d),
                   (w2, w2_d), (ws1, ws1_d), (ws2, ws2_d), (ipad, ipad_d),
                   (bd1, bd1_d), (bd2, bd2_d), (xslabt, xslabt_d)):
        nc.sync.dma_start(t_[:], d_.ap()[:])

    ident = setup.tile([128, 128], BF16)
    make_identity(nc, ident[:])

    # y2 node-major accumulator (L1 output, bf16, 128-padded cols, pre-zeroed)
    y2nm = setup.tile([128, NT, 128], BF16)
    nc.gpsimd.memset(y2nm[:], 0.0)
    # y2 feature-major slab (+ ones row) for L2 self-interaction
    y2t = setup.tile([CQ + 1, SLAB], BF16)
    nc.gpsimd.memset(y2t[:], 1.0)  # row CQ stays 1.0; rows 0..CQ-1 overwritten per node tile
    # final output slab
    outsb = setup.tile([128, NT, OP], F32)

    # DRAM tiles for the collective
    y2slab_dr = dram.tile([SLAB, 128], BF16)
    y2all_dr = dram.tile([NPAD, 128], BF16)

    # ---- transport coefficients (shared by both layers) ----
    # rows: 0=c1, 1=c2, 2=s1, 3=sn1, 4=s2, 5=sn2 ; c_m = sin(m*phi + pi/2)
    mrow = setup.tile([128, 6], F32)
    for j, v in enumerate([1.0, 2.0, 1.0, -1.0, 2.0, -2.0]):
        nc.gpsimd.memset(mrow[:, j:j + 1], v)
    mphi = setup.tile([128, 6, T], F32)
    nc.vector.tensor_tensor(
        out=mphi[:],
        in0=phi[:].unsqueeze(1).to_broadcast([128, 6, T]),
        in1=mrow[:].unsqueeze(2).to_broadcast([128, 6, T]),
        op=ALU.mult,
    )
    pi = float(np.pi)
    # wrap c-rows with +pi/2 folded in, then a second plain wrap for large |phi|
    nc.vector.add_range_wrap(mphi[:, 0:2, :], mphi[:, 0:2, :], pi / 2, pi, 2 * pi)
    nc.vector.add_range_wrap(mphi[:, 2:6, :], mphi[:, 2:6, :], 0.0, pi, 2 * pi)
    nc.vector.add_range_wrap(mphi[:], mphi[:], 0.0, pi, 2 * pi)
    coef = setup.tile([128, 6, T], BF16)
    nc.scalar.activation(coef[:], mphi[:], AF.Sin, scale=1.0)

    def cslice(row, k):
        return coef[:, row, k * CHT:(k + 1) * CHT].unsqueeze(2).to_broadcast(
            [128, CHT, C])

    for layer in range(2):
        table_d = xpad_d.ap()[:] if layer == 0 else y2all_dr[:]
        wmat = w1 if layer == 0 else w2
        wself = ws1 if layer == 0 else ws2
        selfrhs = xslabt if layer == 0 else y2t

        xts = []
        for k in range(NCHUNK):
            xg = gpool.tile([128, CHT, 128], BF16, tag="xg")
            # SWDGE desc ring holds ~1024 descriptors; split the gather
            for o in range(0, CHT * 128, 1024):
                nc.gpsimd.dma_gather(
                    out_ap=xg[:, o // 128:(o + 1024) // 128, :],
                    in_ap=table_d,
                    idxs_ap=idx[:, (k * CHT * 128 + o) // 16:
                                (k * CHT * 128 + o + 1024) // 16],
                    num_idxs=1024,
                    num_idxs_reg=1024,
                    elem_size=128,
                )
            # transport: q0 copy; for m=1,2: out_a = c*a + sn*b ; out_b = s*a + c*b
            xt = xtpool.tile([128, CHT, CQ], BF16, tag="xt")
            nc.vector.tensor_copy(xt[:, :, 0:C], xg[:, :, 0:C])
            for m in (1, 2):
                a = xg[:, :, (2 * m - 1) * C:(2 * m) * C]
                b = xg[:, :, (2 * m) * C:(2 * m + 1) * C]
                cc, ss, sn = cslice(m - 1, k), cslice(2 * m, k), cslice(2 * m + 1, k)
                t1 = tmppool.tile([128, CHT, C], BF16, tag="t1")
                t2 = tmppool.tile([128, CHT, C], BF16, tag="t2")
                nc.vector.tensor_tensor(out=t1[:], in0=a, in1=cc, op=ALU.mult)
                nc.vector.tensor_tensor(out=t2[:], in0=b, in1=sn, op=ALU.mult)
                nc.vector.tensor_tensor(
                    out=xt[:, :, (2 * m - 1) * C:(2 * m) * C], in0=t1[:], in1=t2[:],
                    op=ALU.add)
                t3 = tmppool.tile([128, CHT, C], BF16, tag="t1")
                t4 = tmppool.tile([128, CHT, C], BF16, tag="t2")
                nc.vector.tensor_tensor(out=t3[:], in0=a, in1=ss, op=ALU.mult)
                nc.vector.tensor_tensor(out=t4[:], in0=b, in1=cc, op=ALU.mult)
                nc.vector.tensor_tensor(
                    out=xt[:, :, (2 * m) * C:(2 * m + 1) * C], in0=t3[:], in1=t4[:],
                    op=ALU.add)
            xts.append(xt)

        for nt in range(NT):
            # G windows are 40 fp32 wide; a matmul out cannot cross a 512-fp32
            # psum bank, so split the 32 windows as 12+12+8 across 3 bank tiles.
            Gparts = [Gpool.tile([CQ, 12, WIN], F32, tag="Ga", name="Ga"),
                      Gpool.tile([CQ, 12, WIN], F32, tag="Gb", name="Gb"),
                      Gpool.tile([CQ, 8, WIN], F32, tag="Gc", name="Gc")]
            for g in range(GPNT):
                t = nt * GPNT + g
                k, i = t // CHT, t % CHT
                Gp, gg = Gparts[min(g // 12, 2)], g - 12 * min(g // 12, 2)
                nc.tensor.matmul(Gp[:, gg, :], xts[k][:, i, :], poh[:, t, :],
                                 start=True, stop=True)
            gsb = nsb.tile([CQ, NF, GPNT, GW], BF16, tag="gsb")
            for pi_, (glo, gn) in enumerate(((0, 12), (12, 12), (24, 8))):
                nc.scalar.activation(
                    gsb[:, :, glo:glo + gn, :],
                    Gparts[pi_][:].rearrange("p g (f n) -> p f g n", f=NF, n=GW),
                    AF.Copy)

            agg = npsum.tile([OP, 128], F32, tag="ps")
            for j in range(NF):
                nc.tensor.matmul(agg[:], wmat[:, j, :], gsb[:, j].opt(),
                                 start=(j == 0), stop=False)
            nsl = slice(nt * 128, (nt + 1) * 128)
            nc.tensor.matmul(agg[:], wself[:], selfrhs[:, nsl],
                             start=False, stop=(layer == 0))
            if layer == 1:
                nc.tensor.matmul(agg[:], ipad[:], xslabt[:, nsl],
                                 start=False, stop=True)
            ysb = nsb.tile([OP, 128], BF16, tag="ysb")
            nc.scalar.activation(ysb[:], agg[:], AF.Copy)

            sps = npsum.tile([C * NS, 128], F32, tag="ps")
            nc.tensor.matmul(sps[:], bd1[:], ysb[:], start=True, stop=True)
            ssb = nsb.tile([C * NS, 128], BF16, tag="ssb")
            nc.scalar.activation(ssb[:], sps[:], AF.Relu)

            y2ps = npsum.tile([128, OP], F32, tag="ps")
            nc.tensor.matmul(y2ps[:], ssb[:], bd2[:], start=True, stop=True)
            if layer == 0:
                nc.scalar.activation(y2nm[:, nt, 0:OP], y2ps[:], AF.Copy)
                trps = npsum.tile([OP, 128], BF16, tag="ps")
                nc.tensor.transpose(trps[:], y2nm[:, nt, 0:OP], ident[:])
                nc.scalar.activation(y2t[0:CQ, nsl], trps[:], AF.Copy)
            else:
                nc.scalar.activation(outsb[:, nt, :], y2ps[:], AF.Copy)

        if layer == 0:
            nc.sync.dma_start(
                y2slab_dr[:].rearrange("(t p) f -> p t f", p=128), y2nm[:])
            nc.gpsimd.collective_compute(
                "AllGather", ALU.bypass,
                replica_groups=[list(range(NCORES))],
                ins=[y2slab_dr[:].opt()],
                outs=[y2all_dr[:].opt()],
            )

    nc.sync.dma_start(out_d.ap()[:].rearrange("(t p) f -> p t f", p=128), outsb[:])
    ctx.close()


# ---------------- host-side preprocessing ----------------

def _wrap_idx(idx_flat):
    n = idx_flat.shape[0]
    w = np.zeros((128, n // 16), np.int16)
    blk = idx_flat.reshape(n // 16, 16).T.astype(np.int16)
    for g in range(8):
        w[g * 16:(g + 1) * 16, :] = blk
    return w


def _nonlin_mats():
    theta = (2.0 * np.pi / NS) * np.arange(NS, dtype=np.float64)
    m = np.arange(1, ORDER + 1, dtype=np.float64)
    ang = theta[:, None] * m[None, :]
    cs = np.stack([np.cos(ang), np.sin(ang)], axis=-1).reshape(NS, 2 * ORDER)
    B = np.concatenate([np.ones((NS, 1)), cs], axis=1)          # [NS, Q]
    scale = np.concatenate([[1.0 / NS], np.full(2 * ORDER, 2.0 / NS)])
    # y cols (q*16+c) -> s cols (k*16+c):  BD1[(q*16+c),(k*16+c')] = B[k,q] d_cc'
    bd1 = np.zeros((CQ, C * NS), np.float32)
    bd2 = np.zeros((C * NS, OP), np.float32)
    for c in range(C):
        for k in range(NS):
            for q in range(Q):
                bd1[q * C + c, k * C + c] = B[k, q]
                bd2[k * C + c, q * C + c] = B[k, q] * scale[q]
    return bd1, bd2


def _prep(x, edge_index, pre2d, phi, W1, b1, Ws1, W2, b2, Ws2):
    """Build per-core in_maps (index bookkeeping + layout/dtype prep only)."""
    src = edge_index[:, 0].astype(np.int64)
    dst = edge_index[:, 1].astype(np.int64)

    # node-major (q*16+c) layout, bf16, rows padded to 128 cols
    xq = np.ascontiguousarray(x.transpose(0, 2, 1)).reshape(N_NODES, CQ)
    xpad = np.zeros((NPAD, 128), ml_dtypes.bfloat16)
    xpad[:N_NODES, :CQ] = xq

    order = np.argsort(dst, kind="stable")
    group = (dst // GW)  # global group id 0..2499 (nodes 0..9999)

    def wmat(W):
        return np.ascontiguousarray(
            W.transpose(3, 1, 4, 5, 2, 0).reshape(CQ, NF, OP)).astype(
                ml_dtypes.bfloat16)

    def wsmat(Ws, b):
        m = np.zeros((CQ + 1, OP), np.float32)
        m[:CQ] = Ws.transpose(3, 1, 2, 0).reshape(CQ, OP)
        m[CQ, :C] = b
        return m.astype(ml_dtypes.bfloat16)

    ipad = np.zeros((CQ + 1, OP), np.float32)
    ipad[:CQ, :CQ] = np.eye(CQ)
    bd1, bd2 = _nonlin_mats()

    shared = {
        "xpad": xpad,
        "w1": wmat(W1), "w2": wmat(W2),
        "ws1": wsmat(Ws1, b1), "ws2": wsmat(Ws2, b2),
        "ipad": ipad.astype(ml_dtypes.bfloat16),
        "bd1": bd1.astype(ml_dtypes.bfloat16),
        "bd2": bd2.astype(ml_dtypes.bfloat16),
    }

    in_maps = []
    for c in range(NCORES):
        lo, hi = c * SLAB, (c + 1) * SLAB
        sel = order[(dst[order] >= lo) & (dst[order] < hi)]
        g_loc = group[sel] - c * (SLAB // GW)

        idx_flat = np.zeros(NIDX, np.int64)
        poh_c = np.zeros((128, T, WIN), np.float32)
        phi_c = np.zeros((128, T), np.float32)
        # edges of group t go into tile t (slots 0..count-1)
        counts = np.bincount(g_loc, minlength=T)
        if counts.max() > 128:
            raise RuntimeError(f"group overflow: {counts.max()} edges in a 4-node group")
        slot = np.zeros(len(sel), np.int64)
        cum = np.zeros(T + 1, np.int64)
        cum[1:] = np.cumsum(counts)
        slot = np.arange(len(sel)) - cum[g_loc]
        tgt = g_loc * 128 + slot
        idx_flat[tgt] = src[sel]
        p_, t_ = tgt % 128, tgt // 128
        phi_c[p_, t_] = phi[sel]
        nrel = (dst[sel] - lo) % GW
        for fr in range(NF):
            poh_c[p_, t_, fr * GW + nrel] = pre2d[sel, fr]

        m = dict(shared)
        m["idx"] = _wrap_idx(idx_flat)
        m["poh"] = poh_c.astype(ml_dtypes.bfloat16)
        m["phi"] = phi_c
        xs = np.zeros((CQ + 1, SLAB), np.float32)
        ns = min(hi, N_NODES) - lo
        if ns > 0:
            xs[:CQ, :ns] = xq[lo:lo + ns].T
        xs[CQ] = 1.0
        m["xslabt"] = xs.astype(ml_dtypes.bfloat16)
        in_maps.append(m)
    return in_maps


def kernel(x, edge_index, precomp_neigh_edge, connection, W1, b1, Ws1, W2, b2, Ws2):
    x = np.asarray(x, np.float32)
    pre2d = np.asarray(precomp_neigh_edge, np.float32).reshape(N_EDGES, NF)
    phi = np.asarray(connection, np.float32)
    nc = _CACHE.get("nc")
    if nc is None:
        nc = _CACHE["nc"] = _build_program()
    in_maps = _prep(x, np.asarray(edge_index), pre2d, phi,
                    np.asarray(W1, np.float32), np.asarray(b1, np.float32),
                    np.asarray(Ws1, np.float32),
                    np.asarray(W2, np.float32), np.asarray(b2, np.float32),
                    np.asarray(Ws2, np.float32))
    res = run_bass_kernel_spmd(nc, in_maps, core_ids=list(range(NCORES)))
    _CACHE["last_result"] = res
    _CACHE["last_in_maps"] = in_maps
    full = np.concatenate([res.results[c]["out"] for c in range(NCORES)], axis=0)
    y = full[:N_NODES]                                   # [N, 80] cols q*16+c
    return np.ascontiguousarray(
        y.reshape(N_NODES, Q, C).transpose(0, 2, 1)).astype(np.float32)
